# revision 2
# baseline (speedup 1.0000x reference)
"""Dilated KNN (k=9, dilation=2) over query[4, 8192, 64] on 8 NeuronCores.

Sharding: batch b and query-half h per core (core = 2*b + h). Each core
computes scores s[m, n] = 2*x_m.x_n - |x_n|^2 for its 4096 queries against
all 8192 supports of its batch (same ranking as negated squared euclidean
distance), selects the top-17 per row, and emits indices of ranks
0, 2, ..., 16.

Single-DVE-pass top-k ("iota-stamp"):
  PE   : fp32r hi/lo split matmuls (exact products, fp32 PSUM accumulate)
         MM1: [2ah; 2al] . [bh; bh]          (K=128)
         MM2: [2ah; 1; 1] . [bl; -sqh; -sql] (K=66, drops 2*al.bl ~ 1e-6)
  ACT  : evicts PSUM through a monotone Exp map y = exp(s - 42.8), so the
         fp32 value order equals the score order with uniform absolute
         resolution ~2^-23 in score units.
  Pool : copies a prebuilt u8 iota row over byte 0 of every fp32 y,
         value (255 - li), li = column index within a 256-wide chunk.
         Ranking resolution drops to ~3e-5 score units (fine: adjacent
         top-17 gaps are ~1e-1), and every candidate carries its position.
  DVE  : one max8 per 256-chunk (32/tile) -> 256 candidates with embedded
         positions; 3 merge rounds (max8 + match_replace) give the top-24;
         max_index over the 256 candidates recovers each winner's chunk.
  Decode (batched over all tiles at the end):
         global = ((slot >> 3) << 8) + 255 - (bits & 0xFF).
"""

import sys
import types

import numpy as np

B = 4
N = 8192
C = 64
K_OUT = 9
NQ = N // 2
N_CORES = 8
CHUNK = 256          # max8 scan chunk == stamp period
N_CHUNKS = N // CHUNK
SETUP_CHUNK = 512
N_SETUP_CHUNKS = N // SETUP_CHUNK
NEG_BIG = -1.0e38
EXP_SHIFT = 42.8     # y = exp(s - 42.8); relevant scores s in [-25, 111]

STAMP_MODE = "tcopy"  # "tcopy": gpsimd copy from prebuilt row; "iota": per-tile iota
BLK = 1024            # PSUM eviction block (columns per ACT op)
N_BLK = N // BLK


def _install_ntff_shim():
    """bass_utils imports antenv.axon_hooks for trace=True; the agent image
    lacks it. Register the ctypes-based hook so NTFF profiling works."""
    if "antenv.axon_hooks" in sys.modules:
        return
    try:
        from trn_agent_boot.trn_boot import _ntff_profile_via_ctypes

        hook = _ntff_profile_via_ctypes("/opt/axon/libaxon_pjrt.so")
        m = types.ModuleType("antenv.axon_hooks")
        m.get_axon_ntff_profile_hook = lambda: hook
        sys.modules["antenv.axon_hooks"] = m
    except Exception:
        pass


def build_kernel(nc, n_queries=NQ):
    import concourse.mybir as mybir
    import concourse.tile as tile

    F32 = mybir.dt.float32
    F32R = mybir.dt.float32r
    U32 = mybir.dt.uint32
    U8 = mybir.dt.uint8
    I32 = mybir.dt.int32

    m_tiles = n_queries // 128
    xqT = nc.dram_tensor("xqT", [C, n_queries], F32, kind="ExternalInput")
    xsT = nc.dram_tensor("xsT", [C, N], F32, kind="ExternalInput")
    out = nc.dram_tensor("idx", [n_queries, K_OUT], I32, kind="ExternalOutput")

    with tile.TileContext(nc) as tc:
        with (
            tc.tile_pool(name="const", bufs=1) as constp,
            tc.tile_pool(name="big", bufs=1) as bigp,
        ):
            ones2 = constp.tile([2, SETUP_CHUNK], F32)
            nc.vector.memset(ones2[:, :], 1.0)
            ones64 = constp.tile([64, 1], F32)
            nc.vector.memset(ones64[:, :], 1.0)
            bias_t = constp.tile([128, 1], F32)
            nc.vector.memset(bias_t[:, :], -EXP_SHIFT)
            c3 = constp.tile([128, 1], U32)
            nc.vector.memset(c3[:, :], 3)
            c8 = constp.tile([128, 1], U32)
            nc.vector.memset(c8[:, :], 8)
            c255 = constp.tile([128, 1], U32)
            nc.vector.memset(c255[:, :], 255)
            cFF = constp.tile([128, 1], U32)
            nc.vector.memset(cFF[:, :], 0xFF)

            rhs1 = bigp.tile([128, N], F32R)
            rhs2 = bigp.tile([66, N], F32R)
            lhsT1 = bigp.tile([128, n_queries], F32R)
            lhsT2 = bigp.tile([66, n_queries], F32R)
            vall = bigp.tile([128, m_tiles * 24], F32)
            pall = bigp.tile([128, m_tiles * 24], U32)
            outbuf = bigp.tile([128, m_tiles * K_OUT], U32)
            # prebuilt stamp row: byte value (255 - li), li = idx mod 256
            stamp_src = bigp.tile([128, N // 2], U8)
            nc.gpsimd.iota(
                stamp_src[:, :].rearrange("p (a b) -> p a b", b=CHUNK),
                pattern=[[0, (N // 2) // CHUNK], [-1, CHUNK]],
                base=255,
                channel_multiplier=0,
                allow_small_or_imprecise_dtypes=True,
            )

            with (
                tc.tile_pool(name="stage", bufs=6) as stagep,
                tc.tile_pool(name="dtmp", bufs=3) as dtmp,
                tc.tile_pool(name="psq", bufs=4, space="PSUM") as psqp,
            ):
                # support side first: the main loop's tile 0 needs all of
                # rhs1/rhs2 but only the first query tile of lhsT. Query
                # groups interleave with support chunks; the sq-row tails
                # are emitted as independent phase-B work at the end.
                def emit_support_chunk(cc):
                    sl = slice(cc * SETUP_CHUNK, (cc + 1) * SETUP_CHUNK)
                    sqrow = psqp.tile([1, SETUP_CHUNK], F32, tag="sqrow")
                    bt = stagep.tile([C, SETUP_CHUNK], F32, tag="bt")
                    eng = nc.sync if cc % 2 == 0 else nc.gpsimd
                    eng.dma_start(bt[:, :], xsT.ap()[:, sl])
                    bsq = dtmp.tile([C, SETUP_CHUNK], F32, tag="bsq")
                    nc.gpsimd.tensor_mul(bsq[:, :], bt[:, :], bt[:, :])
                    nc.tensor.matmul(
                        sqrow[0:1, :], ones64[:, :], bsq[:, :], start=True, stop=True
                    )
                    nc.scalar.copy(rhs1[0:64, sl], bt[:, :])  # bh
                    nc.scalar.copy(rhs1[64:128, sl], bt[:, :])  # bh dup
                    nc.vector.scalar_tensor_tensor(
                        rhs2[0:64, sl],
                        rhs1[0:64, sl].bitcast(F32),
                        -1.0,
                        bt[:, :],
                        mybir.AluOpType.mult,
                        mybir.AluOpType.add,
                    )  # bl = b - bh (f32r store)
                    return sqrow

                def emit_sq_tail(cc, sqrow):
                    sl = slice(cc * SETUP_CHUNK, (cc + 1) * SETUP_CHUNK)
                    nsqh = dtmp.tile([1, SETUP_CHUNK], F32R, tag="nsqh")
                    nc.vector.tensor_scalar(
                        nsqh[:, :], sqrow[:, :], -1.0, None, mybir.AluOpType.mult
                    )  # -sqh
                    nc.sync.dma_start(rhs2[64:65, sl], nsqh[:, :])
                    nsql = dtmp.tile([1, SETUP_CHUNK], F32R, tag="nsql")
                    nc.vector.scalar_tensor_tensor(
                        nsql[:, :],
                        sqrow[:, :],
                        -1.0,
                        nsqh[:, :].bitcast(F32),
                        mybir.AluOpType.mult,
                        mybir.AluOpType.subtract,
                    )  # -sql = -sq - (-sqh)
                    nc.scalar.dma_start(rhs2[65:66, sl], nsql[:, :])

                def emit_query_group(g):
                    gsl = slice(g * SETUP_CHUNK, (g + 1) * SETUP_CHUNK)
                    at = stagep.tile([C, SETUP_CHUNK], F32, tag="at")
                    eng = nc.sync if g % 2 == 0 else nc.gpsimd
                    eng.dma_start(at[:, :], xqT.ap()[:, gsl])
                    nc.scalar.mul(lhsT1[0:64, gsl], at[:, :], 2.0)  # 2ah
                    al = dtmp.tile([64, SETUP_CHUNK], F32, tag="al")
                    nc.vector.scalar_tensor_tensor(
                        al[:, :],
                        lhsT1[0:64, gsl].bitcast(F32),
                        -0.5,
                        at[:, :],
                        mybir.AluOpType.mult,
                        mybir.AluOpType.add,
                    )  # a - ah
                    nc.scalar.mul(lhsT1[64:128, gsl], al[:, :], 2.0)  # 2al
                    nc.vector.tensor_copy(lhsT2[0:64, gsl], lhsT1[0:64, gsl])

                for cc in range(N_SETUP_CHUNKS):
                    sqrow = emit_support_chunk(cc)
                    emit_sq_tail(cc, sqrow)
                    if cc % 2 == 1:
                        emit_query_group(cc // 2)
                nc.sync.dma_start(
                    lhsT2[64:66, :]
                    .bitcast(F32)
                    .rearrange("p (r c) -> p r c", c=SETUP_CHUNK),
                    ones2[:, :].unsqueeze(1).broadcast_to(
                        [2, n_queries // SETUP_CHUNK, SETUP_CHUNK]
                    ),
                )

            with (
                tc.tile_pool(name="spool", bufs=2) as spool,
                tc.tile_pool(name="cpool", bufs=2) as cpool,
                tc.tile_pool(name="pmm", bufs=4, space="PSUM") as pmm,
            ):
                # batched decode: global = ((slot>>3)<<8) | (255 - (bits&0xFF))
                # 255 - (bits & 0xFF) == (bits ^ 0xFF) & 0xFF; base has low
                # 8 bits zero so add == bitwise or. Runs in two halves so the
                # first half (and its output DMA) overlaps the main loop.
                base = bigp.tile([128, m_tiles * K_OUT], U32)
                lowb = bigp.tile([128, m_tiles * K_OUT], U32)

                def emit_decode(t0, t1):
                    ts = slice(t0, t1)
                    js = slice(t0 * K_OUT, t1 * K_OUT)
                    base_v = base[:, :].rearrange("p (t j) -> p t j", j=K_OUT)
                    lowb_v = lowb[:, :].rearrange("p (t j) -> p t j", j=K_OUT)
                    pall_v = pall[:, :].rearrange("p (t x) -> p t x", x=24)
                    vbits_v = (
                        vall[:, :]
                        .bitcast(U32)
                        .rearrange("p (t x) -> p t x", x=24)[:, ts, 0:17:2]
                    )
                    nc.vector.tensor_scalar(
                        base_v[:, ts, :],
                        pall_v[:, ts, 0:17:2],
                        c3[:, :],
                        c8[:, :],
                        mybir.AluOpType.logical_shift_right,
                        op1=mybir.AluOpType.logical_shift_left,
                    )
                    nc.vector.tensor_scalar(
                        lowb_v[:, ts, :],
                        vbits_v,
                        cFF[:, :],
                        cFF[:, :],
                        mybir.AluOpType.bitwise_xor,
                        op1=mybir.AluOpType.bitwise_and,
                    )
                    nc.vector.tensor_tensor(
                        outbuf[:, js], base[:, js], lowb[:, js],
                        mybir.AluOpType.bitwise_or,
                    )
                    nc.sync.dma_start(
                        out.ap().rearrange("(t p) j -> p t j", p=128)[:, ts, :],
                        outbuf[:, js].bitcast(I32).rearrange(
                            "p (t j) -> p t j", j=K_OUT
                        ),
                    )

                for t in range(m_tiles):
                    qsl = slice(t * 128, (t + 1) * 128)
                    y = spool.tile([128, N], F32, tag="y")
                    cand = cpool.tile([128, 256], F32, tag="cand")
                    for q in range(N_BLK):
                        pq = pmm.tile([128, BLK], F32, tag="pq")
                        for c in range(BLK // 512):
                            sl = slice(
                                q * BLK + c * 512, q * BLK + (c + 1) * 512
                            )
                            psl = slice(c * 512, (c + 1) * 512)
                            nc.tensor.matmul(
                                pq[:, psl],
                                lhsT1[:, qsl],
                                rhs1[:, sl],
                                start=True,
                                stop=False,
                            )
                            nc.tensor.matmul(
                                pq[:, psl],
                                lhsT2[:, qsl],
                                rhs2[:, sl],
                                start=False,
                                stop=True,
                            )
                        ysl = y[:, q * BLK : (q + 1) * BLK]
                        nc.scalar.activation(
                            ysl,
                            pq[:, :],
                            mybir.ActivationFunctionType.Exp,
                            bias=bias_t[:, :],
                            scale=1.0,
                        )
                    # stamp byte0 of each fp32 with (255 - li), li in 0..255
                    for h in range(2):
                        b0 = (
                            y[:, h * (N // 2) : (h + 1) * (N // 2)]
                            .bitcast(U8)
                            .rearrange("p (n four) -> p n four", four=4)[:, :, 0]
                        )
                        if STAMP_MODE == "tcopy":
                            nc.gpsimd.tensor_copy(b0, stamp_src[:, :])
                        else:
                            nc.gpsimd.iota(
                                b0.rearrange("p (a b) -> p a b", b=CHUNK),
                                pattern=[[0, N_CHUNKS // 2], [-1, CHUNK]],
                                base=255,
                                channel_multiplier=0,
                                allow_small_or_imprecise_dtypes=True,
                            )
                    for ck in range(N_CHUNKS):
                        nc.vector.max(
                            cand[:, ck * 8 : (ck + 1) * 8],
                            y[:, ck * CHUNK : (ck + 1) * CHUNK],
                        )

                    for r in range(3):
                        vsl = slice(t * 24 + r * 8, t * 24 + (r + 1) * 8)
                        nc.vector.max(vall[:, vsl], cand[:, :])
                        nc.vector.max_index(
                            pall[:, t * 24 + r * 8 : t * 24 + (r + 1) * 8],
                            vall[:, vsl],
                            cand[:, :],
                        )
                        if r < 2:
                            nc.vector.match_replace(
                                cand[:, :], vall[:, vsl], cand[:, :], NEG_BIG
                            )
                    if t == m_tiles // 2 - 1:
                        emit_decode(0, m_tiles // 2)
                if True:
                    emit_decode(m_tiles // 2, m_tiles)

    return nc


_COMPILED = None


def _get_compiled():
    global _COMPILED
    if _COMPILED is None:
        _install_ntff_shim()
        import concourse.bacc as bacc

        nc = bacc.Bacc("TRN2", target_bir_lowering=False, debug=False)
        build_kernel(nc)
        nc.compile()
        _COMPILED = nc
    return _COMPILED


LAST_RESULTS = None


def kernel(query: np.ndarray, _trace=False, _tmpdir=None) -> np.ndarray:
    global LAST_RESULTS
    from concourse import bass_utils

    query = np.ascontiguousarray(query, dtype=np.float32)
    assert query.shape == (B, N, C), query.shape
    nc = _get_compiled()

    in_maps = []
    qT = np.ascontiguousarray(query.transpose(0, 2, 1))  # [B, C, N]
    for core in range(N_CORES):
        b, h = divmod(core, 2)
        in_maps.append(
            {
                "xqT": np.ascontiguousarray(qT[b, :, h * NQ : (h + 1) * NQ]),
                "xsT": qT[b],
            }
        )
    res = bass_utils.run_bass_kernel_spmd(
        nc, in_maps, core_ids=list(range(N_CORES)), trace=_trace, tmpdir=_tmpdir
    )
    LAST_RESULTS = res
    out = np.empty((B, N, K_OUT), np.int32)
    for core in range(N_CORES):
        b, h = divmod(core, 2)
        out[b, h * NQ : (h + 1) * NQ, :] = res.results[core]["idx"]
    return out


# revision 3
# speedup vs baseline: 1.7980x; 1.7980x over previous
"""Dilated KNN (k=9, dilation=2) over query[4, 8192, 64] on 8 NeuronCores.

Sharding: batch b and query-half h per core (core = 2*b + h). Each core
computes scores s[m, n] = 2*x_m.x_n - |x_n|^2 for its 4096 queries against
all 8192 supports of its batch (same ranking as negated squared euclidean
distance), selects the top-17 per row, and emits indices of ranks
0, 2, ..., 16.

Single-DVE-pass top-k ("iota-stamp"):
  PE   : fp32r hi/lo split matmuls (exact products, fp32 PSUM accumulate)
         MM1: [2ah; 2al] . [bh; bh]          (K=128)
         MM2: [2ah; 1; 1] . [bl; -sqh; -sql] (K=66, drops 2*al.bl ~ 1e-6)
  ACT  : evicts PSUM through a monotone Exp map y = exp(s - 42.8), so the
         fp32 value order equals the score order with uniform absolute
         resolution ~2^-23 in score units.
  Pool : copies a prebuilt u8 iota row over byte 0 of every fp32 y,
         value (255 - li), li = column index within a 256-wide chunk.
         Ranking resolution drops to ~3e-5 score units (fine: adjacent
         top-17 gaps are ~1e-1), and every candidate carries its position.
  DVE  : one max8 per 256-chunk (32/tile) -> 256 candidates with embedded
         positions; 3 merge rounds (max8 + match_replace) give the top-24;
         max_index over the 256 candidates recovers each winner's chunk.
  Decode (batched over all tiles at the end):
         global = ((slot >> 3) << 8) + 255 - (bits & 0xFF).
"""

import sys
import types

import numpy as np

B = 4
N = 8192
C = 64
K_OUT = 9
NQ = N // 2
N_CORES = 8
CHUNK = 256          # max8 scan chunk == stamp period
N_CHUNKS = N // CHUNK
SETUP_CHUNK = 512
N_SETUP_CHUNKS = N // SETUP_CHUNK
NEG_BIG = -1.0e38
EXP_SHIFT = 42.8     # y = exp(s - 42.8); relevant scores s in [-25, 111]

STAMP_MODE = "iota"  # "tcopy": gpsimd copy from prebuilt row; "iota": per-tile iota
BLK = 1024            # PSUM eviction block (columns per ACT op)
N_BLK = N // BLK


def _install_ntff_shim():
    """bass_utils imports antenv.axon_hooks for trace=True; the agent image
    lacks it. Register the ctypes-based hook so NTFF profiling works."""
    if "antenv.axon_hooks" in sys.modules:
        return
    try:
        from trn_agent_boot.trn_boot import _ntff_profile_via_ctypes

        hook = _ntff_profile_via_ctypes("/opt/axon/libaxon_pjrt.so")
        m = types.ModuleType("antenv.axon_hooks")
        m.get_axon_ntff_profile_hook = lambda: hook
        sys.modules["antenv.axon_hooks"] = m
    except Exception:
        pass


def build_kernel(nc, n_queries=NQ):
    import concourse.mybir as mybir
    import concourse.tile as tile

    F32 = mybir.dt.float32
    F32R = mybir.dt.float32r
    U32 = mybir.dt.uint32
    U8 = mybir.dt.uint8
    I32 = mybir.dt.int32

    m_tiles = n_queries // 128
    xqT = nc.dram_tensor("xqT", [C, n_queries], F32, kind="ExternalInput")
    xsT = nc.dram_tensor("xsT", [C, N], F32, kind="ExternalInput")
    out = nc.dram_tensor("idx", [n_queries, K_OUT], I32, kind="ExternalOutput")

    with tile.TileContext(nc) as tc:
        with (
            tc.tile_pool(name="const", bufs=1) as constp,
            tc.tile_pool(name="big", bufs=1) as bigp,
        ):
            ones2 = constp.tile([2, SETUP_CHUNK], F32)
            nc.vector.memset(ones2[:, :], 1.0)
            ones64 = constp.tile([64, 1], F32)
            nc.vector.memset(ones64[:, :], 1.0)
            bias_t = constp.tile([128, 1], F32)
            nc.vector.memset(bias_t[:, :], -EXP_SHIFT)
            c3 = constp.tile([128, 1], U32)
            nc.vector.memset(c3[:, :], 3)
            c8 = constp.tile([128, 1], U32)
            nc.vector.memset(c8[:, :], 8)
            c255 = constp.tile([128, 1], U32)
            nc.vector.memset(c255[:, :], 255)
            cFF = constp.tile([128, 1], U32)
            nc.vector.memset(cFF[:, :], 0xFF)

            rhs1 = bigp.tile([128, N], F32R)
            rhs2 = bigp.tile([66, N], F32R)
            lhsT1 = bigp.tile([128, n_queries], F32R)
            lhsT2 = bigp.tile([66, n_queries], F32R)
            vall = bigp.tile([128, m_tiles * 24], F32)
            pall = bigp.tile([128, m_tiles * 24], U32)
            outbuf = bigp.tile([128, m_tiles * K_OUT], U32)
            # prebuilt stamp row: byte value (255 - li), li = idx mod 256
            stamp_src = bigp.tile([128, N // 2], U8)
            nc.gpsimd.iota(
                stamp_src[:, :].rearrange("p (a b) -> p a b", b=CHUNK),
                pattern=[[0, (N // 2) // CHUNK], [-1, CHUNK]],
                base=255,
                channel_multiplier=0,
                allow_small_or_imprecise_dtypes=True,
            )

            with (
                tc.tile_pool(name="stage", bufs=6) as stagep,
                tc.tile_pool(name="dtmp", bufs=3) as dtmp,
                tc.tile_pool(name="psq", bufs=4, space="PSUM") as psqp,
            ):
                # support side first: the main loop's tile 0 needs all of
                # rhs1/rhs2 but only the first query tile of lhsT. Query
                # groups interleave with support chunks; the sq-row tails
                # are emitted as independent phase-B work at the end.
                def emit_support_chunk(cc):
                    sl = slice(cc * SETUP_CHUNK, (cc + 1) * SETUP_CHUNK)
                    sqrow = psqp.tile([1, SETUP_CHUNK], F32, tag="sqrow")
                    bt = stagep.tile([C, SETUP_CHUNK], F32, tag="bt")
                    eng = nc.sync if cc % 2 == 0 else nc.gpsimd
                    eng.dma_start(bt[:, :], xsT.ap()[:, sl])
                    bsq = dtmp.tile([C, SETUP_CHUNK], F32, tag="bsq")
                    nc.gpsimd.tensor_mul(bsq[:, :], bt[:, :], bt[:, :])
                    nc.tensor.matmul(
                        sqrow[0:1, :], ones64[:, :], bsq[:, :], start=True, stop=True
                    )
                    nc.scalar.copy(rhs1[0:64, sl], bt[:, :])  # bh
                    nc.scalar.copy(rhs1[64:128, sl], bt[:, :])  # bh dup
                    nc.vector.scalar_tensor_tensor(
                        rhs2[0:64, sl],
                        rhs1[0:64, sl].bitcast(F32),
                        -1.0,
                        bt[:, :],
                        mybir.AluOpType.mult,
                        mybir.AluOpType.add,
                    )  # bl = b - bh (f32r store)
                    return sqrow

                def emit_sq_tail(cc, sqrow):
                    sl = slice(cc * SETUP_CHUNK, (cc + 1) * SETUP_CHUNK)
                    nsqh = dtmp.tile([1, SETUP_CHUNK], F32R, tag="nsqh")
                    nc.vector.tensor_scalar(
                        nsqh[:, :], sqrow[:, :], -1.0, None, mybir.AluOpType.mult
                    )  # -sqh
                    nc.sync.dma_start(rhs2[64:65, sl], nsqh[:, :])
                    nsql = dtmp.tile([1, SETUP_CHUNK], F32R, tag="nsql")
                    nc.vector.scalar_tensor_tensor(
                        nsql[:, :],
                        sqrow[:, :],
                        -1.0,
                        nsqh[:, :].bitcast(F32),
                        mybir.AluOpType.mult,
                        mybir.AluOpType.subtract,
                    )  # -sql = -sq - (-sqh)
                    nc.scalar.dma_start(rhs2[65:66, sl], nsql[:, :])

                def emit_query_group(g):
                    gsl = slice(g * SETUP_CHUNK, (g + 1) * SETUP_CHUNK)
                    at = stagep.tile([C, SETUP_CHUNK], F32, tag="at")
                    eng = nc.sync if g % 2 == 0 else nc.gpsimd
                    eng.dma_start(at[:, :], xqT.ap()[:, gsl])
                    nc.scalar.mul(lhsT1[0:64, gsl], at[:, :], 2.0)  # 2ah
                    al = dtmp.tile([64, SETUP_CHUNK], F32, tag="al")
                    nc.vector.scalar_tensor_tensor(
                        al[:, :],
                        lhsT1[0:64, gsl].bitcast(F32),
                        -0.5,
                        at[:, :],
                        mybir.AluOpType.mult,
                        mybir.AluOpType.add,
                    )  # a - ah
                    nc.scalar.mul(lhsT1[64:128, gsl], al[:, :], 2.0)  # 2al
                    nc.vector.tensor_copy(lhsT2[0:64, gsl], lhsT1[0:64, gsl])

                for cc in range(N_SETUP_CHUNKS):
                    sqrow = emit_support_chunk(cc)
                    emit_sq_tail(cc, sqrow)
                    if cc % 2 == 1:
                        emit_query_group(cc // 2)
                nc.sync.dma_start(
                    lhsT2[64:66, :]
                    .bitcast(F32)
                    .rearrange("p (r c) -> p r c", c=SETUP_CHUNK),
                    ones2[:, :].unsqueeze(1).broadcast_to(
                        [2, n_queries // SETUP_CHUNK, SETUP_CHUNK]
                    ),
                )

            with (
                tc.tile_pool(name="spool", bufs=2) as spool,
                tc.tile_pool(name="cpool", bufs=2) as cpool,
                tc.tile_pool(name="pmm", bufs=4, space="PSUM") as pmm,
            ):
                # batched decode: global = ((slot>>3)<<8) | (255 - (bits&0xFF))
                # 255 - (bits & 0xFF) == (bits ^ 0xFF) & 0xFF; base has low
                # 8 bits zero so add == bitwise or. Runs in two halves so the
                # first half (and its output DMA) overlaps the main loop.
                base = bigp.tile([128, m_tiles * K_OUT], U32)
                lowb = bigp.tile([128, m_tiles * K_OUT], U32)

                def emit_decode(t0, t1):
                    ts = slice(t0, t1)
                    js = slice(t0 * K_OUT, t1 * K_OUT)
                    base_v = base[:, :].rearrange("p (t j) -> p t j", j=K_OUT)
                    lowb_v = lowb[:, :].rearrange("p (t j) -> p t j", j=K_OUT)
                    pall_v = pall[:, :].rearrange("p (t x) -> p t x", x=24)
                    vbits_v = (
                        vall[:, :]
                        .bitcast(U32)
                        .rearrange("p (t x) -> p t x", x=24)[:, ts, 0:17:2]
                    )
                    nc.vector.tensor_scalar(
                        base_v[:, ts, :],
                        pall_v[:, ts, 0:17:2],
                        c3[:, :],
                        c8[:, :],
                        mybir.AluOpType.logical_shift_right,
                        op1=mybir.AluOpType.logical_shift_left,
                    )
                    nc.vector.tensor_scalar(
                        lowb_v[:, ts, :],
                        vbits_v,
                        cFF[:, :],
                        cFF[:, :],
                        mybir.AluOpType.bitwise_xor,
                        op1=mybir.AluOpType.bitwise_and,
                    )
                    nc.vector.tensor_tensor(
                        outbuf[:, js], base[:, js], lowb[:, js],
                        mybir.AluOpType.bitwise_or,
                    )
                    nc.sync.dma_start(
                        out.ap().rearrange("(t p) j -> p t j", p=128)[:, ts, :],
                        outbuf[:, js].bitcast(I32).rearrange(
                            "p (t j) -> p t j", j=K_OUT
                        ),
                    )

                for t in range(m_tiles):
                    qsl = slice(t * 128, (t + 1) * 128)
                    y = spool.tile([128, N], F32, tag="y")
                    cand = cpool.tile([128, 256], F32, tag="cand")
                    for q in range(N_BLK):
                        pq = pmm.tile([128, BLK], F32, tag="pq")
                        for c in range(BLK // 512):
                            sl = slice(
                                q * BLK + c * 512, q * BLK + (c + 1) * 512
                            )
                            psl = slice(c * 512, (c + 1) * 512)
                            nc.tensor.matmul(
                                pq[:, psl],
                                lhsT1[:, qsl],
                                rhs1[:, sl],
                                start=True,
                                stop=False,
                            )
                            nc.tensor.matmul(
                                pq[:, psl],
                                lhsT2[:, qsl],
                                rhs2[:, sl],
                                start=False,
                                stop=True,
                            )
                        ysl = y[:, q * BLK : (q + 1) * BLK]
                        nc.scalar.activation(
                            ysl,
                            pq[:, :],
                            mybir.ActivationFunctionType.Exp,
                            bias=bias_t[:, :],
                            scale=1.0,
                        )
                    # stamp byte0 of each fp32 with (255 - li), li in 0..255
                    for h in range(2):
                        b0 = (
                            y[:, h * (N // 2) : (h + 1) * (N // 2)]
                            .bitcast(U8)
                            .rearrange("p (n four) -> p n four", four=4)[:, :, 0]
                        )
                        if STAMP_MODE == "tcopy":
                            nc.gpsimd.tensor_copy(b0, stamp_src[:, :])
                        else:
                            nc.gpsimd.iota(
                                b0.rearrange("p (a b) -> p a b", b=CHUNK),
                                pattern=[[0, N_CHUNKS // 2], [-1, CHUNK]],
                                base=255,
                                channel_multiplier=0,
                                allow_small_or_imprecise_dtypes=True,
                            )
                    for ck in range(N_CHUNKS):
                        nc.vector.max(
                            cand[:, ck * 8 : (ck + 1) * 8],
                            y[:, ck * CHUNK : (ck + 1) * CHUNK],
                        )

                    for r in range(3):
                        vsl = slice(t * 24 + r * 8, t * 24 + (r + 1) * 8)
                        nc.vector.max(vall[:, vsl], cand[:, :])
                        nc.vector.max_index(
                            pall[:, t * 24 + r * 8 : t * 24 + (r + 1) * 8],
                            vall[:, vsl],
                            cand[:, :],
                        )
                        if r < 2:
                            nc.vector.match_replace(
                                cand[:, :], vall[:, vsl], cand[:, :], NEG_BIG
                            )
                    if t == m_tiles // 2 - 1:
                        emit_decode(0, m_tiles // 2)
                if True:
                    emit_decode(m_tiles // 2, m_tiles)

    return nc


_COMPILED = None


def _get_compiled():
    global _COMPILED
    if _COMPILED is None:
        _install_ntff_shim()
        import concourse.bacc as bacc

        nc = bacc.Bacc("TRN2", target_bir_lowering=False, debug=False)
        build_kernel(nc)
        nc.compile()
        _COMPILED = nc
    return _COMPILED


LAST_RESULTS = None


def kernel(query: np.ndarray, _trace=False, _tmpdir=None) -> np.ndarray:
    global LAST_RESULTS
    from concourse import bass_utils

    query = np.ascontiguousarray(query, dtype=np.float32)
    assert query.shape == (B, N, C), query.shape
    nc = _get_compiled()

    in_maps = []
    qT = np.ascontiguousarray(query.transpose(0, 2, 1))  # [B, C, N]
    for core in range(N_CORES):
        b, h = divmod(core, 2)
        in_maps.append(
            {
                "xqT": np.ascontiguousarray(qT[b, :, h * NQ : (h + 1) * NQ]),
                "xsT": qT[b],
            }
        )
    res = bass_utils.run_bass_kernel_spmd(
        nc, in_maps, core_ids=list(range(N_CORES)), trace=_trace, tmpdir=_tmpdir
    )
    LAST_RESULTS = res
    out = np.empty((B, N, K_OUT), np.int32)
    for core in range(N_CORES):
        b, h = divmod(core, 2)
        out[b, h * NQ : (h + 1) * NQ, :] = res.results[core]["idx"]
    return out


# revision 13
# speedup vs baseline: 1.8667x; 1.0382x over previous
"""Dilated KNN (k=9, dilation=2) over query[4, 8192, 64] on 8 NeuronCores.

Sharding: batch b and query-half h per core (core = 2*b + h). Each core
computes scores s[m, n] = 2*x_m.x_n - |x_n|^2 for its 4096 queries against
all 8192 supports of its batch (same ranking as negated squared euclidean
distance), selects the top-17 per row, and emits indices of ranks
0, 2, ..., 16.

Single-DVE-pass top-k ("iota-stamp"):
  PE   : fp32r hi/lo split matmuls (exact products, fp32 PSUM accumulate)
         MM1: [2ah; 2al] . [bh; bh]          (K=128)
         MM2: [2ah; 1; 1] . [bl; -sqh; -sql] (K=66, drops 2*al.bl ~ 1e-6)
  ACT  : evicts PSUM through a monotone Exp map y = exp(s - 42.8), so the
         fp32 value order equals the score order with uniform absolute
         resolution ~2^-23 in score units.
  Pool : copies a prebuilt u8 iota row over byte 0 of every fp32 y,
         value (255 - li), li = column index within a 256-wide chunk.
         Ranking resolution drops to ~3e-5 score units (fine: adjacent
         top-17 gaps are ~1e-1), and every candidate carries its position.
  DVE  : one max8 per 256-chunk (32/tile) -> 256 candidates with embedded
         positions; 3 merge rounds (max8 + match_replace) give the top-24;
         two max_index calls over the 256 candidates recover the winners'
         chunks for the 9 ranks the dilated output needs (0,2,...,16):
         call 1 uses a stride-2 view of ranks 0..14, call 2 ranks 16..23.
  Decode (batched over all tiles at the end):
         global = ((slot >> 3) << 8) + 255 - (bits & 0xFF).
"""

import sys
import types

import numpy as np

B = 4
N = 8192
C = 64
K_OUT = 9
NQ = N // 2
N_CORES = 8
CHUNK = 256          # max8 scan chunk == stamp period
N_CHUNKS = N // CHUNK
SETUP_CHUNK = 512
N_SETUP_CHUNKS = N // SETUP_CHUNK
NEG_BIG = -1.0e38
EXP_SHIFT = 42.8     # y = exp(s - 42.8); relevant scores s in [-25, 111]

BLK = 2048            # PSUM eviction block (columns per ACT op)
N_BLK = N // BLK


def _install_ntff_shim():
    """bass_utils imports antenv.axon_hooks for trace=True; the agent image
    lacks it. Register the ctypes-based hook so NTFF profiling works."""
    if "antenv.axon_hooks" in sys.modules:
        return
    try:
        from trn_agent_boot.trn_boot import _ntff_profile_via_ctypes

        hook = _ntff_profile_via_ctypes("/opt/axon/libaxon_pjrt.so")
        m = types.ModuleType("antenv.axon_hooks")
        m.get_axon_ntff_profile_hook = lambda: hook
        sys.modules["antenv.axon_hooks"] = m
    except Exception:
        pass


def build_kernel(nc, n_queries=NQ):
    import concourse.mybir as mybir
    import concourse.tile as tile

    F32 = mybir.dt.float32
    F32R = mybir.dt.float32r
    U32 = mybir.dt.uint32
    U8 = mybir.dt.uint8
    I32 = mybir.dt.int32

    m_tiles = n_queries // 128
    xqT = nc.dram_tensor("xqT", [C, n_queries], F32, kind="ExternalInput")
    xsT = nc.dram_tensor("xsT", [C, N], F32, kind="ExternalInput")
    out = nc.dram_tensor("idx", [n_queries, K_OUT], I32, kind="ExternalOutput")

    with tile.TileContext(nc) as tc:
        with (
            tc.tile_pool(name="const", bufs=1) as constp,
            tc.tile_pool(name="big", bufs=1) as bigp,
        ):
            ones2 = constp.tile([2, SETUP_CHUNK], F32)
            nc.vector.memset(ones2[:, :], 1.0)
            ones64 = constp.tile([64, 1], F32)
            nc.vector.memset(ones64[:, :], 1.0)
            bias_t = constp.tile([128, 1], F32)
            nc.vector.memset(bias_t[:, :], -EXP_SHIFT)
            c3 = constp.tile([128, 1], U32)
            nc.vector.memset(c3[:, :], 3)
            c8 = constp.tile([128, 1], U32)
            nc.vector.memset(c8[:, :], 8)
            c255 = constp.tile([128, 1], U32)
            nc.vector.memset(c255[:, :], 255)
            cFF = constp.tile([128, 1], U32)
            nc.vector.memset(cFF[:, :], 0xFF)

            rhs1 = bigp.tile([128, N], F32R)
            rhs2 = bigp.tile([66, N], F32R)
            lhsT1 = bigp.tile([128, n_queries], F32R)
            lhsT2 = bigp.tile([66, n_queries], F32R)
            vall = bigp.tile([128, m_tiles * 24], F32)
            pall = bigp.tile([128, m_tiles * 16], U32)
            outbuf = bigp.tile([128, m_tiles * K_OUT], U32)

            with (
                tc.tile_pool(name="stage", bufs=6) as stagep,
                tc.tile_pool(name="dtmp", bufs=3) as dtmp,
                tc.tile_pool(name="psq", bufs=4, space="PSUM") as psqp,
            ):
                # support side first: the main loop's tile 0 needs all of
                # rhs1/rhs2 but only the first query tile of lhsT. Query
                # groups interleave with support chunks; the sq-row tails
                # are emitted as independent phase-B work at the end.
                def emit_support_chunk(cc):
                    sl = slice(cc * SETUP_CHUNK, (cc + 1) * SETUP_CHUNK)
                    sqrow = psqp.tile([1, SETUP_CHUNK], F32, tag="sqrow")
                    bt = stagep.tile([C, SETUP_CHUNK], F32, tag="bt")
                    eng = nc.sync if cc % 2 == 0 else nc.gpsimd
                    eng.dma_start(bt[:, :], xsT.ap()[:, sl])
                    bsq = dtmp.tile([C, SETUP_CHUNK], F32, tag="bsq")
                    nc.gpsimd.tensor_mul(bsq[:, :], bt[:, :], bt[:, :])
                    nc.tensor.matmul(
                        sqrow[0:1, :], ones64[:, :], bsq[:, :], start=True, stop=True
                    )
                    nc.scalar.copy(rhs1[0:64, sl], bt[:, :])  # bh
                    nc.scalar.copy(rhs1[64:128, sl], bt[:, :])  # bh dup
                    nc.vector.scalar_tensor_tensor(
                        rhs2[0:64, sl],
                        rhs1[0:64, sl].bitcast(F32),
                        -1.0,
                        bt[:, :],
                        mybir.AluOpType.mult,
                        mybir.AluOpType.add,
                    )  # bl = b - bh (f32r store)
                    return sqrow

                def emit_sq_tail(cc, sqrow):
                    sl = slice(cc * SETUP_CHUNK, (cc + 1) * SETUP_CHUNK)
                    nsqh = dtmp.tile([1, SETUP_CHUNK], F32R, tag="nsqh")
                    nc.vector.tensor_scalar(
                        nsqh[:, :], sqrow[:, :], -1.0, None, mybir.AluOpType.mult
                    )  # -sqh
                    nc.sync.dma_start(rhs2[64:65, sl], nsqh[:, :])
                    nsql = dtmp.tile([1, SETUP_CHUNK], F32R, tag="nsql")
                    nc.vector.scalar_tensor_tensor(
                        nsql[:, :],
                        sqrow[:, :],
                        -1.0,
                        nsqh[:, :].bitcast(F32),
                        mybir.AluOpType.mult,
                        mybir.AluOpType.subtract,
                    )  # -sql = -sq - (-sqh)
                    nc.scalar.dma_start(rhs2[65:66, sl], nsql[:, :])

                def emit_query_group(g):
                    gsl = slice(g * SETUP_CHUNK, (g + 1) * SETUP_CHUNK)
                    at = stagep.tile([C, SETUP_CHUNK], F32, tag="at")
                    eng = nc.sync if g % 2 == 0 else nc.gpsimd
                    eng.dma_start(at[:, :], xqT.ap()[:, gsl])
                    nc.scalar.mul(lhsT1[0:64, gsl], at[:, :], 2.0)  # 2ah
                    al = dtmp.tile([64, SETUP_CHUNK], F32, tag="al")
                    nc.vector.scalar_tensor_tensor(
                        al[:, :],
                        lhsT1[0:64, gsl].bitcast(F32),
                        -0.5,
                        at[:, :],
                        mybir.AluOpType.mult,
                        mybir.AluOpType.add,
                    )  # a - ah
                    nc.scalar.mul(lhsT1[64:128, gsl], al[:, :], 2.0)  # 2al
                    nc.vector.tensor_copy(lhsT2[0:64, gsl], lhsT1[0:64, gsl])

                for cc in range(N_SETUP_CHUNKS):
                    sqrow = emit_support_chunk(cc)
                    emit_sq_tail(cc, sqrow)
                    if cc % 2 == 1:
                        emit_query_group(cc // 2)
                nc.sync.dma_start(
                    lhsT2[64:66, :]
                    .bitcast(F32)
                    .rearrange("p (r c) -> p r c", c=SETUP_CHUNK),
                    ones2[:, :].unsqueeze(1).broadcast_to(
                        [2, n_queries // SETUP_CHUNK, SETUP_CHUNK]
                    ),
                )

            with (
                tc.tile_pool(name="spool", bufs=2) as spool,
                tc.tile_pool(name="cpool", bufs=2) as cpool,
                tc.tile_pool(name="pmm", bufs=2, space="PSUM") as pmm,
            ):
                # batched decode: global = ((slot>>3)<<8) | (255 - (bits&0xFF))
                # 255 - (bits & 0xFF) == (bits ^ 0xFF) & 0xFF; base has low
                # 8 bits zero so add == bitwise or. Runs in two halves so the
                # first half (and its output DMA) overlaps the main loop.
                base = bigp.tile([128, m_tiles * K_OUT], U32)
                lowb = bigp.tile([128, m_tiles * K_OUT], U32)

                def emit_decode(t0, t1):
                    ts = slice(t0, t1)
                    js = slice(t0 * K_OUT, t1 * K_OUT)
                    base_v = base[:, :].rearrange("p (t j) -> p t j", j=K_OUT)
                    lowb_v = lowb[:, :].rearrange("p (t j) -> p t j", j=K_OUT)
                    pall_v = pall[:, :].rearrange("p (t x) -> p t x", x=16)
                    vbits_v = (
                        vall[:, :]
                        .bitcast(U32)
                        .rearrange("p (t x) -> p t x", x=24)[:, ts, 0:17:2]
                    )
                    nc.vector.tensor_scalar(
                        base_v[:, ts, :],
                        pall_v[:, ts, 0:K_OUT],
                        c3[:, :],
                        c8[:, :],
                        mybir.AluOpType.logical_shift_right,
                        op1=mybir.AluOpType.logical_shift_left,
                    )
                    nc.vector.tensor_scalar(
                        lowb_v[:, ts, :],
                        vbits_v,
                        cFF[:, :],
                        cFF[:, :],
                        mybir.AluOpType.bitwise_xor,
                        op1=mybir.AluOpType.bitwise_and,
                    )
                    nc.vector.tensor_tensor(
                        outbuf[:, js], base[:, js], lowb[:, js],
                        mybir.AluOpType.bitwise_or,
                    )
                    nc.sync.dma_start(
                        out.ap().rearrange("(t p) j -> p t j", p=128)[:, ts, :],
                        outbuf[:, js].bitcast(I32).rearrange(
                            "p (t j) -> p t j", j=K_OUT
                        ),
                    )

                for t in range(m_tiles):
                    qsl = slice(t * 128, (t + 1) * 128)
                    y = spool.tile([128, N], F32, tag="y")
                    cand = cpool.tile([128, 256], F32, tag="cand")
                    for q in range(N_BLK):
                        pq = pmm.tile([128, BLK], F32, tag="pq")
                        for c in range(BLK // 512):
                            sl = slice(
                                q * BLK + c * 512, q * BLK + (c + 1) * 512
                            )
                            psl = slice(c * 512, (c + 1) * 512)
                            nc.tensor.matmul(
                                pq[:, psl],
                                lhsT1[:, qsl],
                                rhs1[:, sl],
                                start=True,
                                stop=False,
                            )
                            nc.tensor.matmul(
                                pq[:, psl],
                                lhsT2[:, qsl],
                                rhs2[:, sl],
                                start=False,
                                stop=True,
                            )
                        ysl = y[:, q * BLK : (q + 1) * BLK]
                        nc.scalar.activation(
                            ysl,
                            pq[:, :],
                            mybir.ActivationFunctionType.Exp,
                            bias=bias_t[:, :],
                            scale=1.0,
                        )
                    # stamp byte0 of each fp32 with (255 - li), li in 0..255
                    for h in range(2):
                        b0 = (
                            y[:, h * (N // 2) : (h + 1) * (N // 2)]
                            .bitcast(U8)
                            .rearrange("p (n four) -> p n four", four=4)[:, :, 0]
                        )
                        nc.gpsimd.iota(
                            b0.rearrange("p (a b) -> p a b", b=CHUNK),
                            pattern=[[0, N_CHUNKS // 2], [-1, CHUNK]],
                            base=255,
                            channel_multiplier=0,
                            allow_small_or_imprecise_dtypes=True,
                        )
                    for ck in range(N_CHUNKS):
                        nc.vector.max(
                            cand[:, ck * 8 : (ck + 1) * 8],
                            y[:, ck * CHUNK : (ck + 1) * CHUNK],
                        )

                    # 3 extraction rounds; match_replace into fresh buffers so
                    # the original cand stays intact for the index lookups.
                    cand2 = cpool.tile([128, 256], F32, tag="cand2")
                    cand3 = cpool.tile([128, 256], F32, tag="cand3")
                    v0 = slice(t * 24, t * 24 + 8)
                    v1 = slice(t * 24 + 8, t * 24 + 16)
                    v2 = slice(t * 24 + 16, t * 24 + 24)
                    nc.vector.max(vall[:, v0], cand[:, :])
                    nc.vector.match_replace(
                        cand2[:, :], vall[:, v0], cand[:, :], NEG_BIG
                    )
                    nc.vector.max(vall[:, v1], cand2[:, :])
                    nc.vector.match_replace(
                        cand3[:, :], vall[:, v1], cand2[:, :], NEG_BIG
                    )
                    nc.vector.max(vall[:, v2], cand3[:, :])
                    # slots for the 9 needed ranks: {0,2,...,14} then 16..23
                    nc.vector.max_index(
                        pall[:, t * 16 : t * 16 + 8],
                        vall[:, t * 24 : t * 24 + 15 : 2],
                        cand[:, :],
                    )
                    nc.vector.max_index(
                        pall[:, t * 16 + 8 : t * 16 + 16],
                        vall[:, v2],
                        cand[:, :],
                    )
                    if t == m_tiles // 2 - 1:
                        emit_decode(0, m_tiles // 2)
                if True:
                    emit_decode(m_tiles // 2, m_tiles)

    return nc


_COMPILED = None


def _get_compiled():
    global _COMPILED
    if _COMPILED is None:
        _install_ntff_shim()
        import concourse.bacc as bacc

        nc = bacc.Bacc("TRN2", target_bir_lowering=False, debug=False)
        build_kernel(nc)
        nc.compile()
        _COMPILED = nc
    return _COMPILED


LAST_RESULTS = None


def kernel(query: np.ndarray, _trace=False, _tmpdir=None) -> np.ndarray:
    global LAST_RESULTS
    from concourse import bass_utils

    query = np.ascontiguousarray(query, dtype=np.float32)
    assert query.shape == (B, N, C), query.shape
    nc = _get_compiled()

    in_maps = []
    qT = np.ascontiguousarray(query.transpose(0, 2, 1))  # [B, C, N]
    for core in range(N_CORES):
        b, h = divmod(core, 2)
        in_maps.append(
            {
                "xqT": np.ascontiguousarray(qT[b, :, h * NQ : (h + 1) * NQ]),
                "xsT": qT[b],
            }
        )
    res = bass_utils.run_bass_kernel_spmd(
        nc, in_maps, core_ids=list(range(N_CORES)), trace=_trace, tmpdir=_tmpdir
    )
    LAST_RESULTS = res
    out = np.empty((B, N, K_OUT), np.int32)
    for core in range(N_CORES):
        b, h = divmod(core, 2)
        out[b, h * NQ : (h + 1) * NQ, :] = res.results[core]["idx"]
    return out
